# revision 16
# baseline (speedup 1.0000x reference)
"""DCGRU cell on 8 Trainium2 NeuronCores (Bass/Tile), v4.

Decomposition (same sharding as v2/v3)
--------------------------------------
reference: adj2 = adj + I, d_inv = 1/rowsum(adj2), adj_mx = (adj2*d_inv).T,
hop: x_out = adj_mx @ x_in = (d_inv-scaled adj2)^T @ x_in.

Node dim sharded: core m holds the pre-scaled adjacency column strip
SA*d_inv[:,None]*adj2[:, m*1024:(m+1)*1024] SBUF-resident in fp8 and
computes x_out for its 1024 nodes; the thin x operand is re-replicated by
AllGather after each of the 3 producing hops (hop1, xc, hop1c), split in
halves so the collective overlaps compute.

v4 changes vs v3 (185us):
- d_inv*SA folded into the fp8 adjacency on the host: staging a hop
  operand is now transpose + one constant-scale fp8 quantize (single
  quad-wide vector op), no per-node scale vectors on device.
- All PSUM evacuations run on the vector engine (tensor_scalar_mul with
  a constant) so the scalar engine only ever runs Sigmoid/Tanh; both
  ACT tables are preloaded with dummy activations at kernel start
  (each Copy<->Sigmoid<->Tanh switch was a 1.3us ACT_TABLE_LOAD).
- xc staging is decoupled from xcT: transposed r-gate blocks multiply a
  host-precomputed SX0*hx (node-major) straight into the fp8 stage
  buffer; xcT (only needed for the wc0 projection) is built in the AG2
  shadow.
- GRU tail is 2 vector ops: u*hx and (1-u) precompute in the AG shadow,
  tail = (1-u)*c + [u*hx].
- Gather loads split across the sync+scalar DMA queues (2 engines).
- Chebyshev fold + transposed gate/candidate matmuls + PSUM half-split
  hops with early next-AG triggering as in v3.
"""

import sys

if "/opt/trn_rl_repo" not in sys.path:
    sys.path.insert(0, "/opt/trn_rl_repo")

import numpy as np
import ml_dtypes

N = 8192
NCORES = 8
S = N // NCORES          # 1024 nodes per core
D_IN = 2
UNITS = 64
F = D_IN + UNITS         # 66
FP = 80                  # fp8 row pitch (66 padded; dual-fp8 ldweights needs %16)
JBLK = N // 128          # 64 global node blocks
KP = JBLK // 2           # 32 DoubleRow pair blocks
NBLK = S // 128          # 8 local node blocks
HB = NBLK // 2           # 4 blocks per gather half
BF = ml_dtypes.bfloat16
F8 = ml_dtypes.float8_e4m3
SA = 4096.0              # 2**12: scale folded into the fp8 adjacency rows
SX0 = 32.0               # scale on y0 / xc fp8 operands (x ~ O(1))
SXH = 1024.0             # scale on hop-output fp8 operands (~80x smaller)
EV1 = 1.0 / (SX0 * SA)   # hop1/hop1c PSUM evacuation scale
EV2 = 1.0 / (SXH * SA)   # hop2/hop2c PSUM evacuation scale

_CACHE = {}


def _build_and_compile():
    import concourse.bacc as bacc
    import concourse.mybir as mybir
    import concourse.tile as tile
    from concourse import masks

    dt = mybir.dt
    AF = mybir.ActivationFunctionType
    ALU = mybir.AluOpType
    DR = mybir.MatmulPerfMode.DoubleRow
    GROUPS = [list(range(NCORES))]

    nc = bacc.Bacc("TRN2", target_bir_lowering=False, debug=False,
                   num_devices=NCORES)

    adj_d = nc.dram_tensor("adj_s", [N, S], dt.float8e4, kind="ExternalInput")
    y0_d = nc.dram_tensor("y0_full", [128, JBLK * FP], dt.float8e4,
                          kind="ExternalInput")
    x0T_d = nc.dram_tensor("x0T_in", [F + 1, S], dt.bfloat16,
                           kind="ExternalInput")
    hz_d = nc.dram_tensor("hz_in", [128, NBLK * UNITS], dt.float32,
                          kind="ExternalInput")
    stc_d = nc.dram_tensor("stc_in", [128, NBLK * D_IN], dt.float8e4,
                           kind="ExternalInput")
    w0_d = nc.dram_tensor("w0", [F + 1, 2 * UNITS], dt.bfloat16, kind="ExternalInput")
    w1_d = nc.dram_tensor("w1", [F, 2 * UNITS], dt.bfloat16, kind="ExternalInput")
    w2_d = nc.dram_tensor("w2", [F, 2 * UNITS], dt.bfloat16, kind="ExternalInput")
    wc0_d = nc.dram_tensor("wc0", [F + 1, UNITS], dt.bfloat16, kind="ExternalInput")
    wc1_d = nc.dram_tensor("wc1", [F, UNITS], dt.bfloat16, kind="ExternalInput")
    wc2_d = nc.dram_tensor("wc2", [F, UNITS], dt.bfloat16, kind="ExternalInput")
    out_d = nc.dram_tensor("out_loc", [UNITS, S], dt.float32,
                           kind="ExternalOutput")

    warm_in = nc.dram_tensor("warm_in", [128, 4], dt.float32)
    warm_out = nc.dram_tensor("warm_out", [NCORES, 128, 4], dt.float32,
                              addr_space="Shared")
    # 3 gathers x 2 halves, fp8 payload
    st_d = [[nc.dram_tensor(f"st{i}_{h}", [128, HB * FP], dt.float8e4)
             for h in range(2)] for i in range(3)]
    gf_d = [[nc.dram_tensor(f"gf{i}_{h}", [NCORES, 128, HB * FP], dt.float8e4,
                            addr_space="Shared") for h in range(2)]
            for i in range(3)]

    # hop1 follows adjacency DMA arrival (kp ascending); gather-fed hops
    # split pair blocks by the gather half they live in.
    kps_arrival = list(range(KP))
    kps_h = [[c * 4 + h * 2 + j for c in range(NCORES) for j in range(2)]
             for h in range(2)]

    def ypair(y_t, kp):
        # y tiles store node blocks half-major: slot = h*32 + c*4 + k for
        # global block jb = c*8 + h*4 + k, so each gathered half lands as
        # contiguous [128, 8, FP] per-core runs.
        jb = 2 * kp
        c, h, k = jb // NBLK, (jb % NBLK) // HB, jb % HB
        ys = h * 32 + c * HB + k
        return y_t[:, ys:ys + 2, 0:F]

    with tile.TileContext(nc) as tc:
        with (
            tc.tile_pool(name="pers", bufs=1) as pers,
            tc.tile_pool(name="work", bufs=2) as work,
            tc.tile_pool(name="ypool", bufs=2) as ypool,
            tc.tile_pool(name="ps_hop", bufs=2, space="PSUM") as ps_hop,
            tc.tile_pool(name="ps_tr", bufs=2, space="PSUM") as ps_tr,
            tc.tile_pool(name="ps_ru", bufs=2, space="PSUM") as ps_ru,
            tc.tile_pool(name="ps_c", bufs=2, space="PSUM") as ps_c,
        ):
            # CC rail warmup: the first collective's entry sync exits at
            # (launch skew + doorbell time); fire the doorbell as early as
            # the gpsimd startup barrier allows (~11us).
            nc.gpsimd.collective_compute(
                "AllGather", ALU.bypass, replica_groups=GROUPS,
                ins=[warm_in[:]], outs=[warm_out[:]])

            # ---------- bulk DMAs ----------
            y0_sb = ypool.tile([128, JBLK, FP], dt.float8e4, tag="y")
            nc.sync.dma_start(
                y0_sb[:], y0_d.ap().rearrange("p (jb f) -> p jb f", f=FP))
            adj_sb = pers.tile([128, JBLK, S], dt.float8e4, tag="adj")
            # column-phased: all blocks' local-node columns 0:512 arrive
            # first so hop1's ci=0 half (and its AG1a trigger) is gated by
            # 4MB of arrival, not 8MB.
            with nc.named_scope("adj_load"):
                for p2 in range(2):
                    for q in range(16):
                        nc.sync.dma_start(
                            adj_sb[:, 4 * q:4 * q + 4,
                                   p2 * 512:(p2 + 1) * 512],
                            adj_d[q * 512:(q + 1) * 512,
                                  p2 * 512:(p2 + 1) * 512].rearrange(
                                "(jb p) s -> p jb s", p=128))

            hz_sb = pers.tile([128, NBLK, UNITS], dt.float32, tag="hz")
            nc.scalar.dma_start(hz_sb[:], hz_d.ap().rearrange(
                "p (nb u) -> p nb u", u=UNITS))

            w0_sb = pers.tile([F + 1, 2 * UNITS], dt.bfloat16, tag="w0")
            w1_sb = pers.tile([F, 2 * UNITS], dt.bfloat16, tag="w1")
            w2_sb = pers.tile([F, 2 * UNITS], dt.bfloat16, tag="w2")
            wc0_sb = pers.tile([F + 1, UNITS], dt.bfloat16, tag="wc0")
            wc1_sb = pers.tile([F, UNITS], dt.bfloat16, tag="wc1")
            wc2_sb = pers.tile([F, UNITS], dt.bfloat16, tag="wc2")
            for sb, d in [(w0_sb, w0_d), (w1_sb, w1_d), (w2_sb, w2_d),
                          (wc0_sb, wc0_d), (wc1_sb, wc1_d), (wc2_sb, wc2_d)]:
                nc.scalar.dma_start(sb[:], d[:])

            ident_b = pers.tile([128, 128], dt.bfloat16, tag="ident_b")
            masks.make_identity(nc, ident_b[:])

            # preload both ACT tables so no ACT_TABLE_LOAD lands on the
            # critical path (scalar runs only Sigmoid and Tanh)
            dum = work.tile([128, 1], dt.float32, tag="dum")
            nc.gpsimd.memset(dum[:], 0.0)
            dum_s = work.tile([128, 1], dt.float32, tag="dum_s")
            nc.scalar.activation(dum_s[:], dum[:], AF.Sigmoid)
            nc.scalar.activation(dum_s[:], dum[:], AF.Tanh)

            # ---------- persistent intermediates ----------
            # feature order of x0T/xcT: [hx(0:64) | inp(64:66) | ones(66)]
            # (weight rows permuted to match on the host); x1T/x2mT keep
            # y0's staged order, x1cT/x2cm keep xcT's staged order.
            x0T = pers.tile([F + 1, S], dt.bfloat16, tag="x0T")
            x1T = pers.tile([F, S], dt.bfloat16, tag="x1T")
            x2mT = pers.tile([F, S], dt.bfloat16, tag="x2mT")
            xcT = pers.tile([F + 1, S], dt.bfloat16, tag="xcT")
            x1cT = pers.tile([F, S], dt.bfloat16, tag="x1cT")
            nc.scalar.dma_start(x0T[:], x0T_d[:])
            # xc's static rows (inp, ones) come straight from x0T's image
            nc.scalar.dma_start(xcT[UNITS:F + 1, :], x0T_d[UNITS:F + 1, :])

            stage = pers.tile([128, NBLK, FP], dt.float8e4, tag="stage")
            stagec = pers.tile([128, NBLK, FP], dt.float8e4, tag="stagec")
            nc.gpsimd.memset(stage[:], 0.0)
            nc.gpsimd.memset(stagec[:], 0.0)
            # xc's inp columns scaled by SX0, precomputed on host
            nc.scalar.dma_start(stagec[:, :, UNITS:F], stc_d.ap().rearrange(
                "p (nb i) -> p nb i", i=D_IN))
            grT = pers.tile([UNITS, S], dt.bfloat16, tag="grT")
            uT = pers.tile([UNITS, S], dt.float32, tag="uT")
            u1m = pers.tile([UNITS, S], dt.float32, tag="u1m")
            t1u = pers.tile([UNITS, S], dt.float32, tag="t1u")
            outT = pers.tile([UNITS, S], dt.float32, tag="outT")

            def stage_hop(xT_tile, st, gf, ci):
                """transpose + SXH-quantize half ci of a hop output to the
                fp8 stage buffer, then gather."""
                pt4 = ps_tr.tile([128, HB, F], dt.bfloat16, tag="pt4")
                for k in range(HB):
                    nb = ci * HB + k
                    nc.tensor.transpose(
                        pt4[:, k, :], xT_tile[0:F, nb * 128:(nb + 1) * 128],
                        ident_b[0:F, 0:F])
                nc.vector.tensor_scalar_mul(
                    stage[:, ci * HB:(ci + 1) * HB, 0:F], pt4[:], SXH)
                nc.scalar.dma_start(
                    st.ap().rearrange("p (nb f) -> p nb f", f=FP),
                    stage[:, ci * HB:(ci + 1) * HB, :])
                nc.gpsimd.collective_compute(
                    "AllGather", ALU.bypass, replica_groups=GROUPS,
                    ins=[st[:]], outs=[gf[:]])

            def load_half(gf, y_t, h):
                # half h is one contiguous [128, 32*FP] destination; 4
                # chunks across 2 DMA queues for first-chunk MM starts.
                with tc.high_priority():
                    for cq in range(4):
                        eng = (nc.sync, nc.scalar)[cq % 2]
                        eng.dma_start(
                            y_t[:, h * 32 + cq * 8:h * 32 + (cq + 1) * 8,
                                :].rearrange("p (c k) f -> p c (k f)", c=2),
                            gf[cq * 2:(cq + 1) * 2, :, :].rearrange(
                                "c p f -> p c f"))

            # ---------- gconv 1, hop 1 (x1 = M @ x0) ----------
            with nc.named_scope("hop1"):
                for ci in range(2):
                    ph = ps_hop.tile([F, 512], dt.float32, tag="ph")
                    for i, kp in enumerate(kps_arrival):
                        nc.tensor.matmul(
                            ph[:], ypair(y0_sb, kp),
                            adj_sb[:, 2 * kp:2 * kp + 2,
                                   ci * 512:(ci + 1) * 512],
                            start=(i == 0), stop=(i == KP - 1), perf_mode=DR)
                    nc.vector.tensor_scalar_mul(
                        x1T[:, ci * 512:(ci + 1) * 512], ph[:], EV1)
                    stage_hop(x1T, st_d[0][ci], gf_d[0][ci], ci)

            # r-gate x0/x1 partials issue now (PE idle while the CC rail
            # starts up); only the x2m MM waits on hop2.
            pgr = [ps_ru.tile([UNITS, 512], dt.float32, tag="pgr",
                              name=f"pgr{i}") for i in range(2)]
            for ci in range(2):
                half = slice(ci * 512, (ci + 1) * 512)
                nc.tensor.matmul(pgr[ci][:], w0_sb[:, 0:UNITS],
                                 x0T[:, half], start=True, stop=False)
                nc.tensor.matmul(pgr[ci][:], w1_sb[:, 0:UNITS],
                                 x1T[:, half], start=False, stop=False)

            def hop_split(y_t, evac, mid):
                """gather-fed hop: all half-0 pair blocks (both column
                halves) run during the second AG's flight; column half
                ci=0 completes first and evacs (triggering the next AG)
                before ci=1's half-1 blocks run."""
                ph = [ps_hop.tile([F, 512], dt.float32, tag="ph",
                               name=f"ph{i}") for i in range(2)]
                for ci in range(2):
                    for i, kp in enumerate(kps_h[0]):
                        nc.tensor.matmul(
                            ph[ci][:], ypair(y_t, kp),
                            adj_sb[:, 2 * kp:2 * kp + 2,
                                   ci * 512:(ci + 1) * 512],
                            start=(i == 0), stop=False, perf_mode=DR)
                for ci in range(2):
                    for i, kp in enumerate(kps_h[1]):
                        nc.tensor.matmul(
                            ph[ci][:], ypair(y_t, kp),
                            adj_sb[:, 2 * kp:2 * kp + 2,
                                   ci * 512:(ci + 1) * 512],
                            start=False, stop=(i == KP // 2 - 1),
                            perf_mode=DR)
                    evac(ci, ph[ci])
                mid()

            # ---------- gconv 1, hop 2 (x2m = M @ x1; r-gates; xc) ------
            y1 = ypool.tile([128, JBLK, FP], dt.float8e4, tag="y")
            with nc.named_scope("gather1"):
                for h in range(2):
                    load_half(gf_d[0][h], y1, h)

            def evac2(ci, ph):
              with tc.high_priority():
                half = slice(ci * 512, (ci + 1) * 512)
                nc.vector.tensor_scalar_mul(x2mT[:, half], ph[:], EV2)
                nc.tensor.matmul(pgr[ci][:], w2_sb[:, 0:UNITS],
                                 x2mT[:, half], start=False, stop=True)
                nc.scalar.activation(grT[:, half], pgr[ci][:], AF.Sigmoid)
                # staged xc: transposed r blocks * (SX0*hx), node-major
                pt4 = ps_tr.tile([128, HB, F], dt.bfloat16, tag="pt4")
                for k in range(HB):
                    nb = ci * HB + k
                    nc.tensor.transpose(
                        pt4[:, k, 0:UNITS],
                        grT[:, nb * 128:(nb + 1) * 128],
                        ident_b[0:UNITS, 0:UNITS])
                nc.vector.tensor_mul(
                    stagec[:, ci * HB:(ci + 1) * HB, 0:UNITS],
                    pt4[:, :, 0:UNITS], hz_sb[:, ci * HB:(ci + 1) * HB, :])
                nc.scalar.dma_start(
                    st_d[1][ci].ap().rearrange("p (nb f) -> p nb f", f=FP),
                    stagec[:, ci * HB:(ci + 1) * HB, :])
                nc.gpsimd.collective_compute(
                    "AllGather", ALU.bypass, replica_groups=GROUPS,
                    ins=[st_d[1][ci][:]], outs=[gf_d[1][ci][:]])

            pcT = [ps_c.tile([UNITS, 512], dt.float32, tag="pcT",
                             name=f"pcT{i}") for i in range(2)]

            def mid2():
                # AG2 shadow: u-gates (PSUM borrows the ph slots, free
                # until hop1c), xcT build, wc0 candidate partial, and the
                # GRU precomputes u*hx and 1-u.
                for ci in range(2):
                    half = slice(ci * 512, (ci + 1) * 512)
                    pu = ps_hop.tile([F, 512], dt.float32, tag="ph")
                    nc.tensor.matmul(pu[0:UNITS, :], w0_sb[:, UNITS:],
                                     x0T[:, half], start=True, stop=False)
                    nc.tensor.matmul(pu[0:UNITS, :], w1_sb[:, UNITS:],
                                     x1T[:, half], start=False, stop=False)
                    nc.tensor.matmul(pu[0:UNITS, :], w2_sb[:, UNITS:],
                                     x2mT[:, half], start=False, stop=True)
                    nc.scalar.activation(uT[:, half], pu[0:UNITS, :],
                                         AF.Sigmoid)
                    nc.vector.tensor_mul(xcT[0:UNITS, half], grT[:, half],
                                         x0T[0:UNITS, half])
                    nc.tensor.matmul(pcT[ci][:], wc0_sb[:], xcT[:, half],
                                     start=True, stop=False)
                    nc.vector.tensor_mul(t1u[:, half], uT[:, half],
                                         x0T[0:UNITS, half])
                    nc.vector.tensor_scalar(u1m[:, half], uT[:, half],
                                            -1.0, 1.0, op0=ALU.mult,
                                            op1=ALU.add)

            with nc.named_scope("hop2"):
                hop_split(y1, evac2, mid2)

            # ---------- gconv 2, hop 1 (x1c = M @ xc) ----------
            yc = ypool.tile([128, JBLK, FP], dt.float8e4, tag="y")
            with nc.named_scope("gather2"):
                for h in range(2):
                    load_half(gf_d[1][h], yc, h)

            def evac1c(ci, ph):
                with tc.high_priority():
                    nc.vector.tensor_scalar_mul(
                        x1cT[:, ci * 512:(ci + 1) * 512], ph[:], EV1)
                    stage_hop(x1cT, st_d[2][ci], gf_d[2][ci], ci)

            def mid1c():
                # AG3a shadow: wc1 candidate partials
                for ci in range(2):
                    half = slice(ci * 512, (ci + 1) * 512)
                    nc.tensor.matmul(pcT[ci][:], wc1_sb[:], x1cT[:, half],
                                     start=False, stop=False)

            with nc.named_scope("hop1c"):
                hop_split(yc, evac1c, mid1c)

            # ---------- gconv 2, hop 2 (x2cm = M @ x1c; GRU out) --------
            y1c = ypool.tile([128, JBLK, FP], dt.float8e4, tag="y")
            with nc.named_scope("gather3"):
                for h in range(2):
                    load_half(gf_d[2][h], y1c, h)

            def evac2c(ci, ph):
              with tc.high_priority():
                half = slice(ci * 512, (ci + 1) * 512)
                x2c = work.tile([F, 512], dt.bfloat16, tag="x2c")
                nc.vector.tensor_scalar_mul(x2c[:], ph[:], EV2)
                nc.tensor.matmul(pcT[ci][:], wc2_sb[:], x2c[:],
                                 start=False, stop=True)
                c_sb = work.tile([UNITS, 512], dt.float32, tag="c")
                nc.scalar.activation(c_sb[:], pcT[ci][:], AF.Tanh)
                # new^T = (1-u)*c + [u*hx]   (both bracketed terms ready)
                t2 = work.tile([UNITS, 512], dt.float32, tag="t2")
                nc.vector.tensor_mul(t2[:], u1m[:, half], c_sb[:])
                nc.vector.tensor_add(outT[:, half], t2[:], t1u[:, half])
                nc.sync.dma_start(out_d[:, half], outT[:, half])

            with nc.named_scope("hop2c"):
                hop_split(y1c, evac2c, lambda: None)

    nc.compile()
    return nc


def _get_nc():
    if "nc" not in _CACHE:
        _CACHE["nc"] = _build_and_compile()
    return _CACHE["nc"]


PERM = list(range(D_IN, F)) + list(range(D_IN))   # [hx | inp] feature order


def _host_prep(inputs, hx, adj, w_ru, b_ru, w_c, b_c):
    x0 = np.concatenate(
        [np.asarray(inputs, np.float32).reshape(N, D_IN),
         np.asarray(hx, np.float32).reshape(N, UNITS)], axis=1)
    adj = np.asarray(adj, np.float32)
    w_ru = np.asarray(w_ru, np.float32)
    w_c = np.asarray(w_c, np.float32)
    # Chebyshev fold: x2 = 2*M@x1 - x0 -> w0' = w0 - w2, w2' = 2*w2.
    # w0/wc0 rows follow x0T/xcT's [hx | inp | 1] feature order; w1/w2
    # follow the staged order of x1/x2m ([inp | hx]); wc1/wc2 follow the
    # staged order of x1c/x2cm (= xcT's [r*hx | inp]).
    w0 = np.vstack([(w_ru[0::3] - w_ru[2::3])[PERM],
                    np.asarray(b_ru, np.float32)[None, :]]).astype(BF)
    w1 = w_ru[1::3].astype(BF)
    w2 = (2.0 * w_ru[2::3]).astype(BF)
    wc0 = np.vstack([(w_c[0::3] - w_c[2::3])[PERM],
                     np.asarray(b_c, np.float32)[None, :]]).astype(BF)
    wc1 = w_c[1::3][PERM].astype(BF)
    wc2 = (2.0 * w_c[2::3])[PERM].astype(BF)
    diag = np.arange(N)
    d_inv = 1.0 / (1.0 + adj.sum(axis=1, dtype=np.float64))
    # adjacency with +I and SA*d_inv folded in, fp8
    rs = (SA * d_inv)[:, None].astype(np.float32)
    adj_f8 = (adj * rs).astype(F8)
    adj_f8[diag, diag] = ((adj[diag, diag] + 1.0) * rs[:, 0]).astype(F8)
    # y0 = SX0 * x0 in fp8, pitch-FP blocks in slot order
    # (slot = h*32 + c*4 + k for global block jb = c*8 + h*4 + k)
    y0 = np.zeros((N, FP), dtype=np.float32)
    y0[:, 0:F] = SX0 * x0
    perm = [c * NBLK + h * HB + k
            for h in range(2) for c in range(NCORES) for k in range(HB)]
    y0_blk = np.ascontiguousarray(
        y0.astype(F8).reshape(JBLK, 128, FP)[perm].transpose(1, 0, 2).reshape(
            128, JBLK * FP))
    in_maps = []
    for m in range(NCORES):
        sl = slice(m * S, (m + 1) * S)
        hz = (SX0 * x0[sl, D_IN:F]).astype(np.float32)
        stc = (SX0 * x0[sl, 0:D_IN]).astype(F8)
        in_maps.append({
            "adj_s": np.ascontiguousarray(adj_f8[:, sl]),
            "y0_full": y0_blk,
            "x0T_in": np.ascontiguousarray(np.vstack(
                [x0[sl][:, PERM].T, np.ones((1, S), np.float32)]).astype(BF)),
            "hz_in": np.ascontiguousarray(
                hz.reshape(NBLK, 128, UNITS).transpose(1, 0, 2).reshape(
                    128, NBLK * UNITS)),
            "stc_in": np.ascontiguousarray(
                stc.reshape(NBLK, 128, D_IN).transpose(1, 0, 2).reshape(
                    128, NBLK * D_IN)),
            "w0": w0, "w1": w1, "w2": w2,
            "wc0": wc0, "wc1": wc1, "wc2": wc2,
        })
    return in_maps


def _run(in_maps, trace=False):
    from concourse.bass_utils import run_bass_kernel_spmd
    nc = _get_nc()
    res = run_bass_kernel_spmd(nc, in_maps, list(range(NCORES)), trace=trace)
    out = np.concatenate(
        [np.asarray(res.results[m]["out_loc"]).T for m in range(NCORES)],
        axis=0)
    return out.reshape(1, N * UNITS).astype(np.float32), res


def kernel(**inputs):
    in_maps = _host_prep(
        inputs["inputs"], inputs["hx"], inputs["adj"], inputs["w_ru"],
        inputs["b_ru"], inputs["w_c"], inputs["b_c"])
    out, _ = _run(in_maps, trace=False)
    return out


# revision 18
# speedup vs baseline: 1.0263x; 1.0263x over previous
"""DCGRU cell on 8 Trainium2 NeuronCores (Bass/Tile), v4.

Decomposition (same sharding as v2/v3)
--------------------------------------
reference: adj2 = adj + I, d_inv = 1/rowsum(adj2), adj_mx = (adj2*d_inv).T,
hop: x_out = adj_mx @ x_in = (d_inv-scaled adj2)^T @ x_in.

Node dim sharded: core m holds the pre-scaled adjacency column strip
SA*d_inv[:,None]*adj2[:, m*1024:(m+1)*1024] SBUF-resident in fp8 and
computes x_out for its 1024 nodes; the thin x operand is re-replicated by
AllGather after each of the 3 producing hops (hop1, xc, hop1c), split in
halves so the collective overlaps compute.

v4 changes vs v3 (185us):
- d_inv*SA folded into the fp8 adjacency on the host: staging a hop
  operand is now transpose + one constant-scale fp8 quantize (single
  quad-wide vector op), no per-node scale vectors on device.
- All PSUM evacuations run on the vector engine (tensor_scalar_mul with
  a constant) so the scalar engine only ever runs Sigmoid/Tanh; both
  ACT tables are preloaded with dummy activations at kernel start
  (each Copy<->Sigmoid<->Tanh switch was a 1.3us ACT_TABLE_LOAD).
- xc staging is decoupled from xcT: transposed r-gate blocks multiply a
  host-precomputed SX0*hx (node-major) straight into the fp8 stage
  buffer; xcT (only needed for the wc0 projection) is built in the AG2
  shadow.
- GRU tail is 2 vector ops: u*hx and (1-u) precompute in the AG shadow,
  tail = (1-u)*c + [u*hx].
- Gather loads split across the sync+scalar DMA queues (2 engines).
- Chebyshev fold + transposed gate/candidate matmuls + PSUM half-split
  hops with early next-AG triggering as in v3.
"""

import sys

if "/opt/trn_rl_repo" not in sys.path:
    sys.path.insert(0, "/opt/trn_rl_repo")

import numpy as np
import ml_dtypes

N = 8192
NCORES = 8
S = N // NCORES          # 1024 nodes per core
D_IN = 2
UNITS = 64
F = D_IN + UNITS         # 66
FP = 80                  # fp8 row pitch (66 padded; dual-fp8 ldweights needs %16)
JBLK = N // 128          # 64 global node blocks
KP = JBLK // 2           # 32 DoubleRow pair blocks
NBLK = S // 128          # 8 local node blocks
HB = NBLK // 2           # 4 blocks per gather half
BF = ml_dtypes.bfloat16
F8 = ml_dtypes.float8_e4m3
SA = 4096.0              # 2**12: scale folded into the fp8 adjacency rows
SX0 = 32.0               # scale on y0 / xc fp8 operands (x ~ O(1))
SXH = 1024.0             # scale on hop-output fp8 operands (~80x smaller)
EV1 = 1.0 / (SX0 * SA)   # hop1/hop1c PSUM evacuation scale
EV2 = 1.0 / (SXH * SA)   # hop2/hop2c PSUM evacuation scale

_CACHE = {}


def _build_and_compile():
    import concourse.bacc as bacc
    import concourse.mybir as mybir
    import concourse.tile as tile
    from concourse import masks

    dt = mybir.dt
    AF = mybir.ActivationFunctionType
    ALU = mybir.AluOpType
    DR = mybir.MatmulPerfMode.DoubleRow
    GROUPS = [list(range(NCORES))]

    nc = bacc.Bacc("TRN2", target_bir_lowering=False, debug=False,
                   num_devices=NCORES)

    adj_d = nc.dram_tensor("adj_s", [N, S], dt.float8e4, kind="ExternalInput")
    y0_d = nc.dram_tensor("y0_full", [128, JBLK * FP], dt.float8e4,
                          kind="ExternalInput")
    x0T_d = nc.dram_tensor("x0T_in", [F + 1, S], dt.bfloat16,
                           kind="ExternalInput")
    hz_d = nc.dram_tensor("hz_in", [128, NBLK * UNITS], dt.float32,
                          kind="ExternalInput")
    stc_d = nc.dram_tensor("stc_in", [128, NBLK * D_IN], dt.float8e4,
                           kind="ExternalInput")
    w0_d = nc.dram_tensor("w0", [F + 1, 2 * UNITS], dt.bfloat16, kind="ExternalInput")
    w1_d = nc.dram_tensor("w1", [F, 2 * UNITS], dt.bfloat16, kind="ExternalInput")
    w2_d = nc.dram_tensor("w2", [F, 2 * UNITS], dt.bfloat16, kind="ExternalInput")
    wc0_d = nc.dram_tensor("wc0", [F + 1, UNITS], dt.bfloat16, kind="ExternalInput")
    wc1_d = nc.dram_tensor("wc1", [F, UNITS], dt.bfloat16, kind="ExternalInput")
    wc2_d = nc.dram_tensor("wc2", [F, UNITS], dt.bfloat16, kind="ExternalInput")
    out_d = nc.dram_tensor("out_loc", [UNITS, S], dt.float32,
                           kind="ExternalOutput")

    warm_in = nc.dram_tensor("warm_in", [128, 1], dt.float32)
    warm_out = nc.dram_tensor("warm_out", [NCORES, 128, 1], dt.float32,
                              addr_space="Shared")
    # 3 gathers x 2 halves, fp8 payload
    st_d = [[nc.dram_tensor(f"st{i}_{h}", [128, HB * FP], dt.float8e4)
             for h in range(2)] for i in range(3)]
    gf_d = [[nc.dram_tensor(f"gf{i}_{h}", [NCORES, 128, HB * FP], dt.float8e4,
                            addr_space="Shared") for h in range(2)]
            for i in range(3)]

    # hop1 follows adjacency DMA arrival (kp ascending); gather-fed hops
    # split pair blocks by the gather half they live in.
    kps_arrival = list(range(KP))
    kps_h = [[c * 4 + h * 2 + j for c in range(NCORES) for j in range(2)]
             for h in range(2)]

    def ypair(y_t, kp):
        # y tiles store node blocks half-major: slot = h*32 + c*4 + k for
        # global block jb = c*8 + h*4 + k, so each gathered half lands as
        # contiguous [128, 8, FP] per-core runs.
        jb = 2 * kp
        c, h, k = jb // NBLK, (jb % NBLK) // HB, jb % HB
        ys = h * 32 + c * HB + k
        return y_t[:, ys:ys + 2, 0:F]

    with tile.TileContext(nc) as tc:
        with (
            tc.tile_pool(name="pers", bufs=1) as pers,
            tc.tile_pool(name="work", bufs=2) as work,
            tc.tile_pool(name="ypool", bufs=2) as ypool,
            tc.tile_pool(name="ps_hop", bufs=2, space="PSUM") as ps_hop,
            tc.tile_pool(name="ps_tr", bufs=2, space="PSUM") as ps_tr,
            tc.tile_pool(name="ps_ru", bufs=2, space="PSUM") as ps_ru,
            tc.tile_pool(name="ps_c", bufs=2, space="PSUM") as ps_c,
        ):
            # CC rail warmup: the first collective's entry sync exits at
            # (launch skew + doorbell time); fire the doorbell as early as
            # the gpsimd startup barrier allows (~11us).
            nc.gpsimd.collective_compute(
                "AllGather", ALU.bypass, replica_groups=GROUPS,
                ins=[warm_in[:]], outs=[warm_out[:]])

            # ---------- bulk DMAs ----------
            y0_sb = ypool.tile([128, JBLK, FP], dt.float8e4, tag="y")
            nc.sync.dma_start(
                y0_sb[:], y0_d.ap().rearrange("p (jb f) -> p jb f", f=FP))
            adj_sb = pers.tile([128, JBLK, S], dt.float8e4, tag="adj")
            with nc.named_scope("adj_load"):
                for q in range(16):
                    nc.sync.dma_start(
                        adj_sb[:, 4 * q:4 * q + 4, :],
                        adj_d[q * 512:(q + 1) * 512, :].rearrange(
                            "(jb p) s -> p jb s", p=128))

            hz_sb = pers.tile([128, NBLK, UNITS], dt.float32, tag="hz")
            nc.scalar.dma_start(hz_sb[:], hz_d.ap().rearrange(
                "p (nb u) -> p nb u", u=UNITS))

            w0_sb = pers.tile([F + 1, 2 * UNITS], dt.bfloat16, tag="w0")
            w1_sb = pers.tile([F, 2 * UNITS], dt.bfloat16, tag="w1")
            w2_sb = pers.tile([F, 2 * UNITS], dt.bfloat16, tag="w2")
            wc0_sb = pers.tile([F + 1, UNITS], dt.bfloat16, tag="wc0")
            wc1_sb = pers.tile([F, UNITS], dt.bfloat16, tag="wc1")
            wc2_sb = pers.tile([F, UNITS], dt.bfloat16, tag="wc2")
            for sb, d in [(w0_sb, w0_d), (w1_sb, w1_d), (w2_sb, w2_d),
                          (wc0_sb, wc0_d), (wc1_sb, wc1_d), (wc2_sb, wc2_d)]:
                nc.scalar.dma_start(sb[:], d[:])

            ident_b = pers.tile([128, 128], dt.bfloat16, tag="ident_b")
            masks.make_identity(nc, ident_b[:])

            # preload both ACT tables so no ACT_TABLE_LOAD lands on the
            # critical path (scalar runs only Sigmoid and Tanh)
            dum = work.tile([128, 1], dt.float32, tag="dum")
            nc.gpsimd.memset(dum[:], 0.0)
            dum_s = work.tile([128, 1], dt.float32, tag="dum_s")
            nc.scalar.activation(dum_s[:], dum[:], AF.Sigmoid)
            nc.scalar.activation(dum_s[:], dum[:], AF.Tanh)

            # ---------- persistent intermediates ----------
            # feature order of x0T/xcT: [hx(0:64) | inp(64:66) | ones(66)]
            # (weight rows permuted to match on the host); x1T/x2mT keep
            # y0's staged order, x1cT/x2cm keep xcT's staged order.
            x0T = pers.tile([F + 1, S], dt.bfloat16, tag="x0T")
            x1T = pers.tile([F, S], dt.bfloat16, tag="x1T")
            x2mT = pers.tile([F, S], dt.bfloat16, tag="x2mT")
            xcT = pers.tile([F + 1, S], dt.bfloat16, tag="xcT")
            x1cT = pers.tile([F, S], dt.bfloat16, tag="x1cT")
            nc.scalar.dma_start(x0T[:], x0T_d[:])
            # xc's static rows (inp, ones) come straight from x0T's image
            nc.scalar.dma_start(xcT[UNITS:F + 1, :], x0T_d[UNITS:F + 1, :])

            stage = pers.tile([128, NBLK, FP], dt.float8e4, tag="stage")
            stagec = pers.tile([128, NBLK, FP], dt.float8e4, tag="stagec")
            nc.gpsimd.memset(stage[:], 0.0)
            nc.gpsimd.memset(stagec[:], 0.0)
            # xc's inp columns scaled by SX0, precomputed on host
            nc.scalar.dma_start(stagec[:, :, UNITS:F], stc_d.ap().rearrange(
                "p (nb i) -> p nb i", i=D_IN))
            grT = pers.tile([UNITS, S], dt.bfloat16, tag="grT")
            uT = pers.tile([UNITS, S], dt.float32, tag="uT")
            u1m = pers.tile([UNITS, S], dt.float32, tag="u1m")
            t1u = pers.tile([UNITS, S], dt.float32, tag="t1u")
            outT = pers.tile([UNITS, S], dt.float32, tag="outT")

            def stage_hop(xT_tile, st, gf, ci):
                """transpose + SXH-quantize half ci of a hop output to the
                fp8 stage buffer, then gather."""
                pt4 = ps_tr.tile([128, HB, F], dt.bfloat16, tag="pt4")
                for k in range(HB):
                    nb = ci * HB + k
                    nc.tensor.transpose(
                        pt4[:, k, :], xT_tile[0:F, nb * 128:(nb + 1) * 128],
                        ident_b[0:F, 0:F])
                nc.vector.tensor_scalar_mul(
                    stage[:, ci * HB:(ci + 1) * HB, 0:F], pt4[:], SXH)
                nc.scalar.dma_start(
                    st.ap().rearrange("p (nb f) -> p nb f", f=FP),
                    stage[:, ci * HB:(ci + 1) * HB, :])
                nc.gpsimd.collective_compute(
                    "AllGather", ALU.bypass, replica_groups=GROUPS,
                    ins=[st[:]], outs=[gf[:]])

            def load_half(gf, y_t, h):
                # half h is one contiguous [128, 32*FP] destination; 4
                # chunks across 2 DMA queues for first-chunk MM starts.
                with tc.high_priority():
                    for cq in range(4):
                        eng = (nc.sync, nc.scalar)[cq % 2]
                        eng.dma_start(
                            y_t[:, h * 32 + cq * 8:h * 32 + (cq + 1) * 8,
                                :].rearrange("p (c k) f -> p c (k f)", c=2),
                            gf[cq * 2:(cq + 1) * 2, :, :].rearrange(
                                "c p f -> p c f"))

            # ---------- gconv 1, hop 1 (x1 = M @ x0) ----------
            with nc.named_scope("hop1"):
                for ci in range(2):
                    ph = ps_hop.tile([F, 512], dt.float32, tag="ph")
                    for i, kp in enumerate(kps_arrival):
                        nc.tensor.matmul(
                            ph[:], ypair(y0_sb, kp),
                            adj_sb[:, 2 * kp:2 * kp + 2,
                                   ci * 512:(ci + 1) * 512],
                            start=(i == 0), stop=(i == KP - 1), perf_mode=DR)
                    nc.vector.tensor_scalar_mul(
                        x1T[:, ci * 512:(ci + 1) * 512], ph[:], EV1)
                    stage_hop(x1T, st_d[0][ci], gf_d[0][ci], ci)

            # r-gate x0/x1 partials issue now (PE idle while the CC rail
            # starts up); only the x2m MM waits on hop2.
            pgr = [ps_ru.tile([UNITS, 512], dt.float32, tag="pgr",
                              name=f"pgr{i}") for i in range(2)]
            for ci in range(2):
                half = slice(ci * 512, (ci + 1) * 512)
                nc.tensor.matmul(pgr[ci][:], w0_sb[:, 0:UNITS],
                                 x0T[:, half], start=True, stop=False)
                nc.tensor.matmul(pgr[ci][:], w1_sb[:, 0:UNITS],
                                 x1T[:, half], start=False, stop=False)

            def hop_split(y_t, evac, mid):
                """gather-fed hop: all half-0 pair blocks (both column
                halves) run during the second AG's flight; column half
                ci=0 completes first and evacs (triggering the next AG)
                before ci=1's half-1 blocks run."""
                ph = [ps_hop.tile([F, 512], dt.float32, tag="ph",
                               name=f"ph{i}") for i in range(2)]
                for ci in range(2):
                    for i, kp in enumerate(kps_h[0]):
                        nc.tensor.matmul(
                            ph[ci][:], ypair(y_t, kp),
                            adj_sb[:, 2 * kp:2 * kp + 2,
                                   ci * 512:(ci + 1) * 512],
                            start=(i == 0), stop=False, perf_mode=DR)
                for ci in range(2):
                    for i, kp in enumerate(kps_h[1]):
                        nc.tensor.matmul(
                            ph[ci][:], ypair(y_t, kp),
                            adj_sb[:, 2 * kp:2 * kp + 2,
                                   ci * 512:(ci + 1) * 512],
                            start=False, stop=(i == KP // 2 - 1),
                            perf_mode=DR)
                    evac(ci, ph[ci])
                mid()

            # ---------- gconv 1, hop 2 (x2m = M @ x1; r-gates; xc) ------
            y1 = ypool.tile([128, JBLK, FP], dt.float8e4, tag="y")
            with nc.named_scope("gather1"):
                for h in range(2):
                    load_half(gf_d[0][h], y1, h)

            def evac2(ci, ph):
              with tc.high_priority():
                half = slice(ci * 512, (ci + 1) * 512)
                nc.vector.tensor_scalar_mul(x2mT[:, half], ph[:], EV2)
                nc.tensor.matmul(pgr[ci][:], w2_sb[:, 0:UNITS],
                                 x2mT[:, half], start=False, stop=True)
                nc.scalar.activation(grT[:, half], pgr[ci][:], AF.Sigmoid)
                # staged xc: transposed r blocks * (SX0*hx), node-major
                pt4 = ps_tr.tile([128, HB, F], dt.bfloat16, tag="pt4")
                for k in range(HB):
                    nb = ci * HB + k
                    nc.tensor.transpose(
                        pt4[:, k, 0:UNITS],
                        grT[:, nb * 128:(nb + 1) * 128],
                        ident_b[0:UNITS, 0:UNITS])
                nc.vector.tensor_mul(
                    stagec[:, ci * HB:(ci + 1) * HB, 0:UNITS],
                    pt4[:, :, 0:UNITS], hz_sb[:, ci * HB:(ci + 1) * HB, :])
                nc.scalar.dma_start(
                    st_d[1][ci].ap().rearrange("p (nb f) -> p nb f", f=FP),
                    stagec[:, ci * HB:(ci + 1) * HB, :])
                nc.gpsimd.collective_compute(
                    "AllGather", ALU.bypass, replica_groups=GROUPS,
                    ins=[st_d[1][ci][:]], outs=[gf_d[1][ci][:]])

            pcT = [ps_c.tile([UNITS, 512], dt.float32, tag="pcT",
                             name=f"pcT{i}") for i in range(2)]

            def mid2():
                # AG2 shadow: u-gates (PSUM borrows the ph slots, free
                # until hop1c), xcT build, wc0 candidate partial, and the
                # GRU precomputes u*hx and 1-u.
                for ci in range(2):
                    half = slice(ci * 512, (ci + 1) * 512)
                    pu = ps_hop.tile([F, 512], dt.float32, tag="ph")
                    nc.tensor.matmul(pu[0:UNITS, :], w0_sb[:, UNITS:],
                                     x0T[:, half], start=True, stop=False)
                    nc.tensor.matmul(pu[0:UNITS, :], w1_sb[:, UNITS:],
                                     x1T[:, half], start=False, stop=False)
                    nc.tensor.matmul(pu[0:UNITS, :], w2_sb[:, UNITS:],
                                     x2mT[:, half], start=False, stop=True)
                    nc.scalar.activation(uT[:, half], pu[0:UNITS, :],
                                         AF.Sigmoid)
                    nc.vector.tensor_mul(xcT[0:UNITS, half], grT[:, half],
                                         x0T[0:UNITS, half])
                    nc.tensor.matmul(pcT[ci][:], wc0_sb[:], xcT[:, half],
                                     start=True, stop=False)
                    nc.vector.tensor_mul(t1u[:, half], uT[:, half],
                                         x0T[0:UNITS, half])
                    nc.vector.tensor_scalar(u1m[:, half], uT[:, half],
                                            -1.0, 1.0, op0=ALU.mult,
                                            op1=ALU.add)

            with nc.named_scope("hop2"):
                hop_split(y1, evac2, mid2)

            # ---------- gconv 2, hop 1 (x1c = M @ xc) ----------
            yc = ypool.tile([128, JBLK, FP], dt.float8e4, tag="y")
            with nc.named_scope("gather2"):
                for h in range(2):
                    load_half(gf_d[1][h], yc, h)

            def evac1c(ci, ph):
                with tc.high_priority():
                    nc.vector.tensor_scalar_mul(
                        x1cT[:, ci * 512:(ci + 1) * 512], ph[:], EV1)
                    stage_hop(x1cT, st_d[2][ci], gf_d[2][ci], ci)

            def mid1c():
                # AG3a shadow: wc1 candidate partials
                for ci in range(2):
                    half = slice(ci * 512, (ci + 1) * 512)
                    nc.tensor.matmul(pcT[ci][:], wc1_sb[:], x1cT[:, half],
                                     start=False, stop=False)

            with nc.named_scope("hop1c"):
                hop_split(yc, evac1c, mid1c)

            # ---------- gconv 2, hop 2 (x2cm = M @ x1c; GRU out) --------
            y1c = ypool.tile([128, JBLK, FP], dt.float8e4, tag="y")
            with nc.named_scope("gather3"):
                for h in range(2):
                    load_half(gf_d[2][h], y1c, h)

            def evac2c(ci, ph):
              with tc.high_priority():
                half = slice(ci * 512, (ci + 1) * 512)
                x2c = work.tile([F, 512], dt.bfloat16, tag="x2c")
                nc.vector.tensor_scalar_mul(x2c[:], ph[:], EV2)
                nc.tensor.matmul(pcT[ci][:], wc2_sb[:], x2c[:],
                                 start=False, stop=True)
                c_sb = work.tile([UNITS, 512], dt.float32, tag="c")
                nc.scalar.activation(c_sb[:], pcT[ci][:], AF.Tanh)
                # new^T = (1-u)*c + [u*hx]   (both bracketed terms ready)
                t2 = work.tile([UNITS, 512], dt.float32, tag="t2")
                nc.vector.tensor_mul(t2[:], u1m[:, half], c_sb[:])
                nc.vector.tensor_add(outT[:, half], t2[:], t1u[:, half])
                nc.sync.dma_start(out_d[:, half], outT[:, half])

            with nc.named_scope("hop2c"):
                hop_split(y1c, evac2c, lambda: None)

    nc.compile()
    return nc


def _get_nc():
    if "nc" not in _CACHE:
        _CACHE["nc"] = _build_and_compile()
    return _CACHE["nc"]


PERM = list(range(D_IN, F)) + list(range(D_IN))   # [hx | inp] feature order


def _host_prep(inputs, hx, adj, w_ru, b_ru, w_c, b_c):
    x0 = np.concatenate(
        [np.asarray(inputs, np.float32).reshape(N, D_IN),
         np.asarray(hx, np.float32).reshape(N, UNITS)], axis=1)
    adj = np.asarray(adj, np.float32)
    w_ru = np.asarray(w_ru, np.float32)
    w_c = np.asarray(w_c, np.float32)
    # Chebyshev fold: x2 = 2*M@x1 - x0 -> w0' = w0 - w2, w2' = 2*w2.
    # w0/wc0 rows follow x0T/xcT's [hx | inp | 1] feature order; w1/w2
    # follow the staged order of x1/x2m ([inp | hx]); wc1/wc2 follow the
    # staged order of x1c/x2cm (= xcT's [r*hx | inp]).
    w0 = np.vstack([(w_ru[0::3] - w_ru[2::3])[PERM],
                    np.asarray(b_ru, np.float32)[None, :]]).astype(BF)
    w1 = w_ru[1::3].astype(BF)
    w2 = (2.0 * w_ru[2::3]).astype(BF)
    wc0 = np.vstack([(w_c[0::3] - w_c[2::3])[PERM],
                     np.asarray(b_c, np.float32)[None, :]]).astype(BF)
    wc1 = w_c[1::3][PERM].astype(BF)
    wc2 = (2.0 * w_c[2::3])[PERM].astype(BF)
    diag = np.arange(N)
    d_inv = 1.0 / (1.0 + adj.sum(axis=1, dtype=np.float64))
    # adjacency with +I and SA*d_inv folded in, fp8
    rs = (SA * d_inv)[:, None].astype(np.float32)
    adj_f8 = (adj * rs).astype(F8)
    adj_f8[diag, diag] = ((adj[diag, diag] + 1.0) * rs[:, 0]).astype(F8)
    # y0 = SX0 * x0 in fp8, pitch-FP blocks in slot order
    # (slot = h*32 + c*4 + k for global block jb = c*8 + h*4 + k)
    y0 = np.zeros((N, FP), dtype=np.float32)
    y0[:, 0:F] = SX0 * x0
    perm = [c * NBLK + h * HB + k
            for h in range(2) for c in range(NCORES) for k in range(HB)]
    y0_blk = np.ascontiguousarray(
        y0.astype(F8).reshape(JBLK, 128, FP)[perm].transpose(1, 0, 2).reshape(
            128, JBLK * FP))
    in_maps = []
    for m in range(NCORES):
        sl = slice(m * S, (m + 1) * S)
        hz = (SX0 * x0[sl, D_IN:F]).astype(np.float32)
        stc = (SX0 * x0[sl, 0:D_IN]).astype(F8)
        in_maps.append({
            "adj_s": np.ascontiguousarray(adj_f8[:, sl]),
            "y0_full": y0_blk,
            "x0T_in": np.ascontiguousarray(np.vstack(
                [x0[sl][:, PERM].T, np.ones((1, S), np.float32)]).astype(BF)),
            "hz_in": np.ascontiguousarray(
                hz.reshape(NBLK, 128, UNITS).transpose(1, 0, 2).reshape(
                    128, NBLK * UNITS)),
            "stc_in": np.ascontiguousarray(
                stc.reshape(NBLK, 128, D_IN).transpose(1, 0, 2).reshape(
                    128, NBLK * D_IN)),
            "w0": w0, "w1": w1, "w2": w2,
            "wc0": wc0, "wc1": wc1, "wc2": wc2,
        })
    return in_maps


def _run(in_maps, trace=False):
    from concourse.bass_utils import run_bass_kernel_spmd
    nc = _get_nc()
    res = run_bass_kernel_spmd(nc, in_maps, list(range(NCORES)), trace=trace)
    out = np.concatenate(
        [np.asarray(res.results[m]["out_loc"]).T for m in range(NCORES)],
        axis=0)
    return out.reshape(1, N * UNITS).astype(np.float32), res


def kernel(**inputs):
    in_maps = _host_prep(
        inputs["inputs"], inputs["hx"], inputs["adj"], inputs["w_ru"],
        inputs["b_ru"], inputs["w_c"], inputs["b_c"])
    out, _ = _run(in_maps, trace=False)
    return out
